# revision 8
# baseline (speedup 1.0000x reference)
"""Causal multi-head attention on 8 Trainium2 NeuronCores.

Problem: B=4, T=2048, C=1024, H=16 heads (head_dim 64), causal softmax,
out = softmax(QK^T/8, causal) V projected by Wo, plus bias.

Sharding (hardcoded): 8 cores = 4 batches x 2 head-groups.  Core c handles
batch b = c//2 and heads g*8..g*8+7 where g = c%2 (tensor parallel over
heads: column-split Wq/Wk/Wv, row-split Wo).  Each core returns a partial
output [T, C]; the host sums the two head-group partials per batch and adds
the bias.

Device algorithm (per core), all in "transposed domain" so no on-chip
transposes are needed:
  xT [C, T] arrives head-dim-major (host passes x[b].T).
  qT = Wq_g^T x^T, kT = Wk_g^T x^T   [512, T]  (dims-on-partitions)
  v  = x Wv_g                        [T, 512]  (tokens-on-partitions)
  per head pair, per 512-wide q block, per 128-wide key tile:
    S^T = kT_h^T qT_h  (keys on partitions, two heads row-packed in the
          128x128 PE array via tile_position)
    E = exp(S^T / 8)  on ScalarE (PSUM -> SBUF bf16), causal-masked on the
        diagonal tiles with gpsimd.affine_select
    ctx^T[h] (+= v_tile^T E) via PE with a staggered ones column appended to
        v so that row 64+h of the PSUM tile accumulates sum_keys E = softmax
        denominators.
  denominators are gathered by DMA, inverted on VectorE, broadcast back by
  DMA, and ctx^T is normalized and cast to bf16.
  partial = ctx^T^T Wo_g accumulated over the 4 head-pair K blocks.

Matmuls run in bf16 (inputs rounded once on device); accumulation is fp32
in PSUM.
"""

import numpy as np

import concourse.bass as bass
import concourse.mybir as mybir
import concourse.tile as tile
from concourse import bacc
from concourse.bass_utils import run_bass_kernel_spmd

F32 = mybir.dt.float32
BF16 = mybir.dt.bfloat16
AF = mybir.ActivationFunctionType

C = 1024
KP = C // 128  # k-tiles over the model dim


def build(S=2048, npair=4):
    """Emit the per-core program.  S = sequence length, npair = head pairs
    (the real problem uses S=2048, npair=4 -> 8 heads, 512 dims per core)."""
    CD = npair * 128        # q/k/v dims owned by this core
    HPC = npair * 2         # heads per core
    NJB = S // 512          # q blocks
    NMT = S // 128          # token tiles

    nc = bacc.Bacc("TRN2", target_bir_lowering=False, debug=False)
    xT = nc.dram_tensor("xT", [C, S], F32, kind="ExternalInput").ap()
    wq = nc.dram_tensor("wq", [C, CD], F32, kind="ExternalInput").ap()
    wk = nc.dram_tensor("wk", [C, CD], F32, kind="ExternalInput").ap()
    wv = nc.dram_tensor("wv", [C, CD], F32, kind="ExternalInput").ap()
    wo = nc.dram_tensor("wo", [CD, C], F32, kind="ExternalInput").ap()
    out = nc.dram_tensor("out", [S, C], F32, kind="ExternalOutput").ap()

    with tile.TileContext(nc) as tc:
        # ---- load x and cast to bf16 (staging pool released early) ----
        with tc.tile_pool(name="cpool", bufs=1) as cpool:
            xT_bf = [cpool.tile([128, S], BF16, name=f"xTb{i}", tag=f"xTb{i}")
                     for i in range(KP)]
            wq_bf = [cpool.tile([128, CD], BF16, name=f"wqb{i}", tag=f"wqb{i}")
                     for i in range(KP)]
            wk_bf = [cpool.tile([128, CD], BF16, name=f"wkb{i}", tag=f"wkb{i}")
                     for i in range(KP)]
            wv_bf = [cpool.tile([128, CD], BF16, name=f"wvb{i}", tag=f"wvb{i}")
                     for i in range(KP)]
            wo_bf = [cpool.tile([128, C], BF16, name=f"wob{j}", tag=f"wob{j}")
                     for j in range(npair)]
            qT_bf = [cpool.tile([128, S], BF16, name=f"qTb{p}", tag=f"qTb{p}")
                     for p in range(npair)]
            kT_bf = [cpool.tile([128, S], BF16, name=f"kTb{p}", tag=f"kTb{p}")
                     for p in range(npair)]
            # v tiles: per head 65 columns [v_h (64) | staggered ones col],
            # ones column of head h sits at local col 64+h via zero padding:
            # layout per head block of 73 cols: v(64), then onehot(h) over 9.
            VW = 64 + HPC + 1  # per-head block width (<= 73)
            v_bf = [cpool.tile([128, HPC * VW], BF16, name=f"vb{m}", tag=f"vb{m}")
                    for m in range(NMT)]
            ctxT_bf = [cpool.tile([128, S], BF16, name=f"cxb{p}", tag=f"cxb{p}")
                       for p in range(npair)]

            with tc.tile_pool(name="ldx", bufs=2) as ldx:
                for i in range(KP):
                    xs = ldx.tile([128, S], F32, name="xs", tag="xs")
                    nc.sync.dma_start(out=xs, in_=xT[i * 128:(i + 1) * 128, :])
                    nc.vector.tensor_copy(xT_bf[i], xs)

            with tc.tile_pool(name="ldw", bufs=3) as ldw:
                for (w_d, w_b) in ((wq, wq_bf), (wk, wk_bf), (wv, wv_bf)):
                    for i in range(KP):
                        ws = ldw.tile([128, CD], F32, name="ws", tag="ws")
                        nc.sync.dma_start(out=ws, in_=w_d[i * 128:(i + 1) * 128, :])
                        nc.gpsimd.tensor_copy(w_b[i], ws)
                for j in range(npair):
                    for h2 in range(2):
                        ws2 = ldw.tile([128, 512], F32, name="ws2", tag="ws2")
                        nc.sync.dma_start(
                            out=ws2,
                            in_=wo[j * 128:(j + 1) * 128, h2 * 512:(h2 + 1) * 512])
                        nc.gpsimd.tensor_copy(wo_bf[j][:, h2 * 512:(h2 + 1) * 512],
                                              ws2)
                # pre-fill v tiles: zeros everywhere, staggered ones columns
                for m in range(NMT):
                    nc.gpsimd.memset(v_bf[m], 0.0)
                    for h in range(HPC):
                        nc.gpsimd.memset(
                            v_bf[m][:, h * VW + 64 + h:h * VW + 65 + h], 1.0)

            # ---- main pipeline: projections interleaved with attention ----
            with tc.tile_pool(name="psum", bufs=1, space="PSUM") as pp, \
                 tc.tile_pool(name="epool", bufs=3) as epool, \
                 tc.tile_pool(name="cstp", bufs=5) as cstp, \
                 tc.tile_pool(name="srp", bufs=1) as srp, \
                 tc.tile_pool(name="smp", bufs=2) as smp, \
                 tc.tile_pool(name="drp", bufs=2, space="DRAM") as drp, \
                 tc.tile_pool(name="rbp", bufs=3) as rbp:

                for jb in range(NJB):
                    # -- projections for this token block --
                    for p in range(npair):
                        for (wb, dstT, nm) in ((wq_bf, qT_bf, "pq"),
                                               (wk_bf, kT_bf, "pk")):
                            ps = pp.tile([128, 512], F32, name=nm, tag="pj", bufs=2)
                            for i in range(KP):
                                nc.tensor.matmul(
                                    ps,
                                    lhsT=wb[i][:, p * 128:(p + 1) * 128],
                                    rhs=xT_bf[i][:, jb * 512:(jb + 1) * 512],
                                    start=(i == 0), stop=(i == KP - 1))
                            nc.vector.tensor_copy(
                                dstT[p][:, jb * 512:(jb + 1) * 512], ps)
                    for m in range(4 * jb, 4 * jb + 4):
                        psv = pp.tile([128, 512], F32, name="psv", tag="pj", bufs=2)
                        for i in range(KP):
                            nc.tensor.matmul(
                                psv[:, 0:CD],
                                lhsT=xT_bf[i][:, m * 128:(m + 1) * 128],
                                rhs=wv_bf[i],
                                start=(i == 0), stop=(i == KP - 1))
                        vv = v_bf[m].rearrange("p (h x) -> p h x", x=VW)
                        nc.vector.tensor_copy(
                            vv[:, :, 0:64],
                            psv[:, 0:CD].rearrange("p (h x) -> p h x", x=64))

                    # -- attention for this q block --
                    sums_row = srp.tile([VW, HPC * 512], F32,
                                        name="sums_row", tag="sr")
                    css = []
                    for p in range(npair):
                        h0, h1 = 2 * p, 2 * p + 1
                        c0 = pp.tile([128, 512], F32, name="c0", tag="pc", bufs=2)
                        c1 = pp.tile([128, 512], F32, name="c1", tag="pc", bufs=2)
                        nkt = 4 * (jb + 1)
                        for kt in range(nkt):
                            ps2 = pp.tile([128, 1024], F32, name="ps2", tag="pa",
                                          bufs=2)
                            nc.tensor.matmul(
                                ps2[:, 0:512],
                                lhsT=kT_bf[p][0:64, kt * 128:(kt + 1) * 128],
                                rhs=qT_bf[p][0:64, jb * 512:(jb + 1) * 512],
                                start=True, stop=True)
                            nc.tensor.matmul(
                                ps2[:, 512:1024],
                                lhsT=kT_bf[p][64:128, kt * 128:(kt + 1) * 128],
                                rhs=qT_bf[p][64:128, jb * 512:(jb + 1) * 512],
                                start=True, stop=True, tile_position=(64, 0))
                            et = epool.tile([128, 1024], BF16, name="et", tag="et")
                            nc.scalar.activation(et, ps2, AF.Exp, scale=0.125)
                            if kt * 128 >= jb * 512:
                                base = jb * 512 - kt * 128
                                for hh in range(2):
                                    nc.gpsimd.affine_select(
                                        out=et[:, hh * 512:(hh + 1) * 512],
                                        in_=et[:, hh * 512:(hh + 1) * 512],
                                        pattern=[[1, 512]],
                                        compare_op=mybir.AluOpType.is_ge,
                                        fill=0.0, base=base,
                                        channel_multiplier=-1)
                            nc.tensor.matmul(
                                c0[0:VW, :],
                                lhsT=v_bf[kt][:, h0 * VW:(h0 + 1) * VW],
                                rhs=et[:, 0:512],
                                start=(kt == 0), stop=(kt == nkt - 1))
                            nc.tensor.matmul(
                                c1[0:VW, :],
                                lhsT=v_bf[kt][:, h1 * VW:(h1 + 1) * VW],
                                rhs=et[:, 512:1024],
                                start=(kt == 0), stop=(kt == nkt - 1))
                        # stage ctx^T and the denominator rows
                        cs = cstp.tile([128, 512], F32, name="cs", tag="cs")
                        nc.vector.tensor_copy(cs[0:64, :], c0[0:64, :])
                        nc.vector.tensor_copy(cs[64:128, :], c1[0:64, :])
                        nc.vector.tensor_copy(
                            sums_row[64:VW, h0 * 512:(h0 + 1) * 512],
                            c0[64:VW, :])
                        nc.vector.tensor_copy(
                            sums_row[64:VW, h1 * 512:(h1 + 1) * 512],
                            c1[64:VW, :])
                        css.append(cs)
                    # gather denominators -> [HPC, 512], invert, broadcast back
                    sums8 = smp.tile([HPC, 512], F32, name="sums8", tag="s8")
                    for h in range(HPC):
                        nc.sync.dma_start(
                            out=sums8[h:h + 1, :],
                            in_=sums_row[64 + h:65 + h, h * 512:(h + 1) * 512])
                    rr8 = smp.tile([HPC, 512], F32, name="rr8", tag="r8")
                    nc.vector.reciprocal(rr8, sums8)
                    rd = drp.tile([HPC, 512], F32, name="rd", tag="rd")
                    nc.sync.dma_start(out=rd, in_=rr8)
                    for p in range(npair):
                        r64 = rbp.tile([128, 512], F32, name="r64", tag="r64")
                        src = bass.AP(rd.tensor, rd.offset + (2 * p) * 512,
                                      [[512, 2], [0, 64], [1, 512]])
                        nc.sync.dma_start(out=r64, in_=src)
                        nc.vector.tensor_mul(
                            ctxT_bf[p][:, jb * 512:(jb + 1) * 512], css[p], r64)

            # ---- output projection ----
            with tc.tile_pool(name="pout", bufs=4, space="PSUM") as pout, \
                 tc.tile_pool(name="obuf", bufs=3) as obuf:
                for mt in range(NMT):
                    for nh in range(2):
                        pso = pout.tile([128, 512], F32, name="pso", tag="po")
                        for j in range(npair):
                            nc.tensor.matmul(
                                pso,
                                lhsT=ctxT_bf[j][:, mt * 128:(mt + 1) * 128],
                                rhs=wo_bf[j][:, nh * 512:(nh + 1) * 512],
                                start=(j == 0), stop=(j == npair - 1))
                        ot = obuf.tile([128, 512], F32, name="ot", tag="ot")
                        nc.scalar.copy(ot, pso)
                        nc.sync.dma_start(
                            out=out[mt * 128:(mt + 1) * 128,
                                    nh * 512:(nh + 1) * 512],
                            in_=ot)

    nc.compile()
    return nc


_NC_CACHE = {}


def _get_nc(S=2048, npair=4):
    key = (S, npair)
    if key not in _NC_CACHE:
        _NC_CACHE[key] = build(S, npair)
    return _NC_CACHE[key]


def make_in_maps(x, Wq, Wk, Wv, Wo):
    """Host-side sharding: batch x head-group slices, x transposed to
    dims-major layout."""
    in_maps = []
    for c in range(8):
        b, g = divmod(c, 2)
        sl = slice(g * 512, (g + 1) * 512)
        in_maps.append({
            "xT": np.ascontiguousarray(x[b].T),
            "wq": np.ascontiguousarray(Wq[:, sl]),
            "wk": np.ascontiguousarray(Wk[:, sl]),
            "wv": np.ascontiguousarray(Wv[:, sl]),
            "wo": np.ascontiguousarray(Wo[sl, :]),
        })
    return in_maps


def run_cores(x, Wq, Wk, Wv, Wo, trace=False, trace_kwargs=None):
    nc = _get_nc(2048, 4)
    in_maps = make_in_maps(x, Wq, Wk, Wv, Wo)
    return run_bass_kernel_spmd(
        nc, in_maps, core_ids=list(range(8)), trace=trace,
        trace_kwargs=trace_kwargs or {})


def kernel(x, Wq, Wk, Wv, Wo, bo):
    x = np.asarray(x, dtype=np.float32)
    Wq = np.asarray(Wq, dtype=np.float32)
    Wk = np.asarray(Wk, dtype=np.float32)
    Wv = np.asarray(Wv, dtype=np.float32)
    Wo = np.asarray(Wo, dtype=np.float32)
    bo = np.asarray(bo, dtype=np.float32)

    res = run_cores(x, Wq, Wk, Wv, Wo).results
    out = np.empty((4, 2048, 1024), dtype=np.float32)
    for b in range(4):
        out[b] = res[2 * b]["out"] + res[2 * b + 1]["out"] + bo[None, :]
    return out
